# revision 4
# baseline (speedup 1.0000x reference)
"""Multi-head attention (S=4096, D=2048, H=16) on 8 trn2 NeuronCores.

Sharding: tensor-parallel by heads — core c computes heads 2c, 2c+1
(columns [256c : 256c+256] of the output), then the host concatenates.
No collectives: every core reads the full (transposed, fp16-cast)
activations and its own weight column-slice.

Schedule (v4): deterministic position-based weaving, no synthetic clock.
The exp pipeline is 13 quarter-unit buffers deep ([128, 8, 512] fp16
each); units u0..u2 + quarter (3,0) are produced during the K/V
projection phases, and each even attention section hosts the next four
quarters — so every exp tile completes ~4 sections before its consuming
section and ACT
(1038ns/pair, the second-busiest engine) never back-pressures the PE
through the 2-buffer PSUM scores pool.  Within a section (64 attnV
matmuls of 129 rows), scores pairs are emitted every 4th sk-tile and
Q-projection chunks (8/section, 4-section windows per group; group 2
runs 16/section in sections 0-1) every 4th sk-tile offset by 2, keeping
pair spacing >= ~1.07us >= the ACT drain rate.

Per-core dataflow (all matmuls fp16 with fp32 PSUM accumulation):
  qT_h[hd, sq]  = sum_c Wq[c-chunk, hd].T @ queryT[c-chunk, sq]   (PE)
  scoresT[sk, sq] = kT_tile.T @ qT -> exp (ACT, scale=1/sqrt(hd)) -> fp16
  out[sq, hd+1] = sum_sk expT_tile.T @ [v | 1]  (ones col => denom)
  out = out[:, :hd] * (1/denom) + bv            (DVE)
Softmax skips max-subtraction: scores ~ N(0,1), exp stays in fp16 range.
"""
import os
import sys

# Make the concourse/Bass stack importable without shadowing an already
# active tree (the axon site dir ships a matched copy and is usually on
# sys.path already; /opt/trn_rl_repo is the fallback).
if not any(os.path.isdir(os.path.join(p, "concourse")) for p in sys.path if p):
    for _p in ("/root/.axon_site/_ro/trn_rl_repo", "/opt/trn_rl_repo"):
        if os.path.isdir(_p):
            sys.path.append(_p)
            break

import numpy as np

S = 4096
D = 2048
HD = 128            # head dim
NCORES = 8
HPC = 2             # heads per core
DH = HPC * HD       # 256 output columns per core
SQ = 512            # seq-group (matmul moving free dim)
G = S // SQ
DC = D // 128       # contraction chunks
SKT = S // 128      # key tiles
TG = SQ // 128      # q sub-tiles per group

_CACHE = {}


def _build_nc(s=S, d=D, reps=1):
    """Build + compile the per-core Bass program (SPMD: same program, 8 cores).

    reps>1 repeats the whole computation inside one NEFF (timing use only)."""
    from concourse import bacc, tile
    import concourse.mybir as mybir

    fp32, fp16 = mybir.dt.float32, mybir.dt.float16
    Exp = mybir.ActivationFunctionType.Exp
    Alu = mybir.AluOpType

    g_, dc, skt = s // SQ, d // 128, s // 128
    scale = float(1.0 / np.sqrt(HD))

    nc = bacc.Bacc("TRN2", target_bir_lowering=False, debug=False,
                   num_devices=NCORES)

    xT = {n: nc.dram_tensor(n, [128, dc, s], fp16, kind="ExternalInput").ap()
          for n in ("qT", "kT", "vT")}
    W = {n: nc.dram_tensor(n, [128, dc, DH], fp16, kind="ExternalInput").ap()
         for n in ("Wq", "Wk", "Wv")}
    bqk_d = nc.dram_tensor("bqk", [128, 2 * HPC], fp32, kind="ExternalInput").ap()
    bvr_d = nc.dram_tensor("bvr", [128, DH], fp32, kind="ExternalInput").ap()
    out_d = nc.dram_tensor("out", [s, DH], fp32, kind="ExternalOutput").ap()

    with tile.TileContext(nc) as tc:
        with (
            tc.tile_pool(name="const", bufs=1) as constp,
            tc.tile_pool(name="wts", bufs=3) as wpool,
            tc.tile_pool(name="persist", bufs=1) as pers,
            tc.tile_pool(name="stg", bufs=2) as stgp,
            tc.tile_pool(name="qs", bufs=6) as qsp,
            tc.tile_pool(name="expp", bufs=13) as expp,
            tc.tile_pool(name="outp", bufs=2) as outp,
            tc.tile_pool(name="small", bufs=4) as smallp,
            tc.tile_pool(name="psA", bufs=2, space="PSUM") as psA,
            tc.tile_pool(name="psO", bufs=3, space="PSUM") as psO,
            tc.tile_pool(name="psQ", bufs=1, space="PSUM") as psQ,
        ):
            # critical-path first: smallest-possible first pieces of Wq and
            # the g0 qT stage so the first projection matmul starts ~2.5us
            # in; remaining pieces sized to the proj consumption rate and
            # interleaved; bias loads trail (first needed ~20us in).
            wq0 = wpool.tile([128, dc, DH], fp16, tag="w", name="Wq")
            stg00 = stgp.tile([128, dc, SQ], fp16, tag="stg")
            nc.sync.dma_start(wq0[:, 0:2, :], W["Wq"][:, 0:2, :])
            nc.sync.dma_start(stg00[:, 0:2, :], xT["qT"][:, 0:2, 0:SQ])
            nc.sync.dma_start(wq0[:, 2:6, :], W["Wq"][:, 2:6, :])
            nc.sync.dma_start(stg00[:, 2:5, :], xT["qT"][:, 2:5, 0:SQ])
            nc.sync.dma_start(wq0[:, 6:11, :], W["Wq"][:, 6:11, :])
            nc.sync.dma_start(stg00[:, 5:8, :], xT["qT"][:, 5:8, 0:SQ])
            nc.sync.dma_start(wq0[:, 11:16, :], W["Wq"][:, 11:16, :])
            nc.sync.dma_start(stg00[:, 8:12, :], xT["qT"][:, 8:12, 0:SQ])
            nc.sync.dma_start(stg00[:, 12:16, :], xT["qT"][:, 12:16, 0:SQ])
            bqk_sb = constp.tile([128, 2 * HPC], fp32, tag="bqk")
            nc.sync.dma_start(bqk_sb[:], bqk_d[:])
            bvr_sb = constp.tile([128, DH], fp32, tag="bvr")
            nc.sync.dma_start(bvr_sb[:], bvr_d[:])
            zero_b = constp.tile([128, 1], fp32, tag="zb")
            nc.vector.memset(zero_b[:], 0.0)

            kTt = [pers.tile([128, s], fp16, tag=f"kT{h}", name=f"kTt{h}")
                   for h in range(HPC)]
            vaug = [pers.tile([128, skt, 129], fp16, tag=f"va{h}", name=f"vaug{h}")
                    for h in range(HPC)]
            for h in range(HPC):
                nc.vector.memset(vaug[h][:, :, 128:129], 1.0)

            def load_w(name, chunks=1):
                w = wpool.tile([128, dc, DH], fp16, tag="w", name=name)
                cs = dc // chunks
                for i in range(chunks):
                    nc.sync.dma_start(w[:, i * cs:(i + 1) * cs, :],
                                      W[name][:, i * cs:(i + 1) * cs, :])
                return w

            def load_stage(name, g, chunks=1):
                stg = stgp.tile([128, dc, SQ], fp16, tag="stg")
                cs = dc // chunks
                for i in range(chunks):
                    nc.sync.dma_start(
                        stg[:, i * cs:(i + 1) * cs, :],
                        xT[name][:, i * cs:(i + 1) * cs, g * SQ:(g + 1) * SQ])
                return stg

            n_units = 2 * g_
            n_sec = 2 * n_units

            for _rep in range(reps):
                exps = {}           # (unit, quarter) -> [128, 8, SQ] fp16 tile
                qs = {}             # g -> [h] -> [128, SQ] fp16 tile
                pend = []           # (unit, quarter, lp) FIFO

                def new_quarter(k, qf):
                    exps[(k, qf)] = expp.tile([128, skt // 4, SQ], fp16,
                                              tag="expT",
                                              name=f"e{k}_{qf}_{_rep}")
                    return exps[(k, qf)]

                def emit_pair(k, qf, lp):
                    """scores for sk-tiles (2gp, 2gp+1) -> exp into 1/4 tile."""
                    g, h = k // 2, k % 2
                    gp = qf * 4 + lp
                    ps = psA.tile([128, 2, SQ], fp32, tag="pA")
                    for j in range(2):
                        nc.tensor.matmul(
                            ps[:, j, :],
                            kTt[h][:, (2 * gp + j) * 128:(2 * gp + j + 1) * 128],
                            qs[g][h][:],
                            start=True, stop=True)
                    nc.scalar.activation(exps[(k, qf)][:, 2 * lp:2 * lp + 2, :],
                                         ps[:], Exp,
                                         bias=zero_b[:, 0:1], scale=scale)

                def emit_next():
                    if pend:
                        emit_pair(*pend.pop(0))

                def flush_pairs_upto(k, qf):
                    while pend and (pend[0][0], pend[0][1]) <= (k, qf):
                        emit_pair(*pend.pop(0))

                def proj_group2(w, stg, dsts, bias_cols, pairs_at=None):
                    """Project both heads c-chunk-interleaved (consumes each
                    stage chunk as it lands; 2 concurrent psO accumulators).
                    pairs_at: {c: (k, hf, lp)} scores pairs for pacing."""
                    pos = [psO.tile([128, 512], fp32, tag="pO",
                                    name=f"pj{h}_{_rep}_{len(exps)}_{len(qs)}")
                           for h in range(HPC)]
                    for c in range(dc):
                        for h in range(HPC):
                            nc.tensor.matmul(pos[h][:],
                                             w[:, c, h * HD:(h + 1) * HD],
                                             stg[:, c, :],
                                             start=(c == 0), stop=(c == dc - 1))
                        if pairs_at and c in pairs_at:
                            emit_pair(*pairs_at[c])
                    for h in range(HPC):
                        nc.vector.tensor_scalar_add(dsts[h], pos[h][:],
                                                    bias_cols[h])

                def qproj_group(wq, g, chunks=1):
                    stg = load_stage("qT", g, chunks=chunks)
                    qs[g] = [qsp.tile([128, SQ], fp16, tag=f"qs{h}",
                                      name=f"qs{h}_{g}_{_rep}")
                             for h in range(HPC)]
                    proj_group2(wq, stg, [qs[g][h][:] for h in range(HPC)],
                                [bqk_sb[:, h:h + 1] for h in range(HPC)])

                # ---- phase A: Wq + Qproj(g0, g1) ----
                if _rep == 0:
                    wq = wq0
                    qs[0] = [qsp.tile([128, SQ], fp16, tag=f"qs{h}",
                                      name=f"qs{h}_0_{_rep}")
                             for h in range(HPC)]
                    proj_group2(wq, stg00,
                                [qs[0][h][:] for h in range(HPC)],
                                [bqk_sb[:, h:h + 1] for h in range(HPC)])
                else:
                    wq = load_w("Wq", chunks=8)
                    qproj_group(wq, 0, chunks=8)
                stg01 = stgp.tile([128, dc, SQ], fp16, tag="stg")
                nc.sync.dma_start(stg01[:, 0:6, :],
                                  xT["qT"][:, 0:6, SQ:2 * SQ])
                nc.sync.dma_start(stg01[:, 6:11, :],
                                  xT["qT"][:, 6:11, SQ:2 * SQ])
                nc.sync.dma_start(stg01[:, 11:16, :],
                                  xT["qT"][:, 11:16, SQ:2 * SQ])
                qs[1] = [qsp.tile([128, SQ], fp16, tag=f"qs{h}",
                                  name=f"qs{h}_1_{_rep}")
                         for h in range(HPC)]
                proj_group2(wq, stg01, [qs[1][h][:] for h in range(HPC)],
                            [bqk_sb[:, h:h + 1] for h in range(HPC)])

                # ---- phase K: Kproj, paced scores pairs for u0/u1 ----
                wk = load_w("Wk", chunks=4)
                for k in range(HPC):
                    for qf in range(4):
                        new_quarter(k, qf)

                for y in range(g_):
                    stg = load_stage("kT", y, chunks=4)
                    if y > 0:   # pairs for y-1 (kT slice written last y)
                        yp = y - 1
                        pa = {3 + 4 * dd: (dd // 2, (2 * yp + dd % 2) // 4,
                                           (2 * yp + dd % 2) % 4)
                              for dd in range(4)}
                    else:
                        pa = None
                    proj_group2(wk, stg,
                                [kTt[h][:, y * SQ:(y + 1) * SQ]
                                 for h in range(HPC)],
                                [bqk_sb[:, HPC + h:HPC + h + 1]
                                 for h in range(HPC)], pa)

                # ---- phase V: Vproj + evenly paced scores (u2, (3,0)) ----
                wv = load_w("Wv", chunks=4)
                for qf in range(4):
                    new_quarter(2, qf)
                new_quarter(3, 0)
                yl = g_ - 1
                vpend = [(h, (2 * yl + dd) // 4, (2 * yl + dd) % 4)
                         for h in range(HPC) for dd in range(2)]
                vpend += [(2, p // 4, p % 4) for p in range(16)]
                vpend += [(3, 0, lp) for lp in range(4)]
                for y in range(g_):
                    stg = load_stage("vT", y, chunks=2)
                    for t in range(TG):
                        po = psO.tile([128, 512], fp32, tag="pO")
                        ps = po[:, 0:DH]
                        for c in range(dc):
                            nc.tensor.matmul(ps[:],
                                             stg[:, c, t * 128:(t + 1) * 128],
                                             wv[:, c, :],
                                             start=(c == 0), stop=(c == dc - 1))
                        for h in range(HPC):
                            nc.vector.tensor_copy(
                                vaug[h][:, y * TG + t, 0:128],
                                ps[:, h * HD:(h + 1) * HD])
                        if vpend:
                            emit_pair(*vpend.pop(0))
                while vpend:
                    emit_pair(*vpend.pop(0))

                # ---- attention sections ----
                # qproj chunk windows: group 2 in sections 0-1 (16/sec),
                # groups 3..7 in sections (4g-10 .. 4g-7) at 8/sec.
                qwin = {}
                for ga in range(2, g_):
                    chunks = [(ga, hh, c) for hh in range(HPC)
                              for c in range(dc)]
                    if ga == 2:
                        secs, per = (0, 1), dc
                    else:
                        w0 = 4 * ga - 10
                        secs, per = (w0, w0 + 1, w0 + 2, w0 + 3), dc // 2
                    for si, mm in enumerate(secs):
                        qwin.setdefault(mm, []).extend(
                            chunks[si * per:(si + 1) * per])
                # qT stage loads: group 2 at end of V phase (above handled
                # here at section -1 == just before sections), others one
                # section before their window.
                qstage_at = {4 * ga - 11: ga for ga in range(3, g_)}

                qp_stage = {2: load_stage("qT", 2, chunks=2)}
                psq = {}

                def qchunk(ga, hh, c):
                    if c == 0:
                        if hh == 0:
                            qs[ga] = [qsp.tile([128, SQ], fp16, tag=f"qs{h2}",
                                               name=f"qs{h2}_{ga}_{_rep}")
                                      for h2 in range(HPC)]
                        psq[(ga, hh)] = psQ.tile([128, SQ], fp32, tag="pQ",
                                                 name=f"psq{hh}_{ga}_{_rep}")
                    nc.tensor.matmul(
                        psq[(ga, hh)][:],
                        wq[:, c, hh * HD:(hh + 1) * HD],
                        qp_stage[ga][:, c, :],
                        start=(c == 0), stop=(c == dc - 1))
                    if c == dc - 1:
                        nc.vector.tensor_scalar_add(
                            qs[ga][hh][:], psq[(ga, hh)][:],
                            bqk_sb[:, hh:hh + 1])

                nextQ = 13          # global quarter index 4*k + qf
                for m in range(n_sec):
                    j, hf = divmod(m, 2)
                    g, h = j // 2, j % 2
                    flush_pairs_upto(j, 3)   # safety net; normally a no-op

                    # host four quarters at each even section (slots freed
                    # by the unit that died at the end of section m-1)
                    if m >= 2 and m % 2 == 0:
                        for _q in range(4):
                            if nextQ < 4 * n_units:
                                k2, qf2 = divmod(nextQ, 4)
                                new_quarter(k2, qf2)
                                pend.extend([(k2, qf2, lp)
                                             for lp in range(4)])
                                nextQ += 1

                    if m in qstage_at:
                        ga2 = qstage_at[m]
                        qp_stage[ga2] = load_stage("qT", ga2, chunks=2)

                    qcs = qwin.get(m, [])
                    dbl = len(qcs) > 8           # group-2 sections: 2/slot
                    # pair slots: 8/section when qproj fills the section,
                    # else 6 (spacing >= ACT drain rate)
                    if qcs:
                        pslots = frozenset(range(3, 32, 4))
                    else:
                        pslots = frozenset((3, 7, 11, 19, 23, 27))

                    pos = [psO.tile([128, 512], fp32, tag="pO",
                                    name=f"po{hf}{t}_{j}_{_rep}")
                           for t in range(2)]

                    def finish_t(tt, hf=hf, g=g, h=h, pos=pos):
                        """normalize + out DMA for one 128-row tile."""
                        t = 2 * hf + tt
                        srcp = pos[tt]
                        rec = smallp.tile([128, 1], fp32, tag="rec")
                        nc.vector.reciprocal(rec[:], srcp[:, 128:129])
                        osb = outp.tile([128, HD], fp32, tag="osb")
                        nc.vector.scalar_tensor_tensor(
                            osb[:], srcp[:, 0:HD], rec[:, 0:1],
                            bvr_sb[:, h * HD:(h + 1) * HD],
                            Alu.mult, Alu.add)
                        nc.sync.dma_start(
                            out_d[g * SQ + t * 128: g * SQ + (t + 1) * 128,
                                  h * HD:(h + 1) * HD],
                            osb[:])

                    if m == n_sec - 1:
                        # final section t-major: the first tile's normalize
                        # and out-DMA drain under the second tile's matmuls,
                        # shortening the end-of-kernel tail
                        for tt in range(2):
                            t = 2 * hf + tt
                            for i in range(skt):
                                nc.tensor.matmul(
                                    pos[tt][:, 0:129],
                                    exps[(j, i // 8)][:, i % 8,
                                                      t * 128:(t + 1) * 128],
                                    vaug[h][:, i, 0:129],
                                    start=(i == 0), stop=(i == skt - 1))
                            finish_t(tt)
                        continue

                    qi = 0
                    for i in range(skt):
                        for tt in range(2):
                            t = 2 * hf + tt
                            nc.tensor.matmul(
                                pos[tt][:, 0:129],
                                exps[(j, i // 8)][:, i % 8,
                                                  t * 128:(t + 1) * 128],
                                vaug[h][:, i, 0:129],
                                start=(i == 0), stop=(i == skt - 1))
                        if i % 4 == 1 and qi < len(qcs):
                            qchunk(*qcs[qi])
                            qi += 1
                            if dbl and qi < len(qcs):
                                qchunk(*qcs[qi])
                                qi += 1
                        if i in pslots:
                            emit_next()
                    while qi < len(qcs):
                        qchunk(*qcs[qi])
                        qi += 1

                    for tt in range(2):         # normalize + out (t-pair)
                        finish_t(tt)

    nc.compile()
    return nc


def _get_nc(s=S, d=D):
    key = (s, d)
    if key not in _CACHE:
        _CACHE[key] = _build_nc(s, d)
    return _CACHE[key]


def _prep_xT(x16):
    """[s, d] fp16 -> [128, d//128, s] contiguous (d-major chunks on partitions)."""
    s, d = x16.shape
    return np.ascontiguousarray(
        x16.T.reshape(d // 128, 128, s).transpose(1, 0, 2))


def _prep_w(w16):
    """[d, DH] fp16 -> [128, d//128, DH] contiguous."""
    d, dh = w16.shape
    return np.ascontiguousarray(
        w16.reshape(d // 128, 128, dh).transpose(1, 0, 2))


def _make_in_maps(query, key_in, value, Wq, bq, Wk, bk, Wv, bv):
    f32 = np.float32
    q16 = np.asarray(query, f32).astype(np.float16)
    k16 = np.asarray(key_in, f32).astype(np.float16)
    v16 = np.asarray(value, f32).astype(np.float16)
    qT, kT, vT = _prep_xT(q16), _prep_xT(k16), _prep_xT(v16)
    Wq = np.asarray(Wq, f32)
    Wk = np.asarray(Wk, f32)
    Wv = np.asarray(Wv, f32)
    bq = np.asarray(bq, f32)
    bk = np.asarray(bk, f32)
    bv = np.asarray(bv, f32)

    in_maps = []
    for c in range(NCORES):
        sl = slice(c * DH, (c + 1) * DH)
        bqk = np.empty((128, 2 * HPC), f32)
        for h in range(HPC):
            bqk[:, h] = bq[sl][h * HD:(h + 1) * HD]
            bqk[:, HPC + h] = bk[sl][h * HD:(h + 1) * HD]
        in_maps.append({
            "qT": qT, "kT": kT, "vT": vT,
            "Wq": _prep_w(Wq[:, sl].astype(np.float16)),
            "Wk": _prep_w(Wk[:, sl].astype(np.float16)),
            "Wv": _prep_w(Wv[:, sl].astype(np.float16)),
            "bqk": bqk,
            "bvr": np.ascontiguousarray(np.tile(bv[sl][None, :], (128, 1))),
        })
    return in_maps


def kernel(query, key_in, value, Wq, bq, Wk, bk, Wv, bv):
    from concourse.bass_utils import run_bass_kernel_spmd

    nc = _get_nc()
    in_maps = _make_in_maps(query, key_in, value, Wq, bq, Wk, bk, Wv, bv)
    # The first execution after device bring-up occasionally fails with a
    # transient NRT_EXEC_UNIT_UNRECOVERABLE — retry before giving up.
    last_exc = None
    for _ in range(3):
        try:
            res = run_bass_kernel_spmd(nc, in_maps, list(range(NCORES)))
            break
        except Exception as exc:  # noqa: BLE001 — retried, then re-raised
            last_exc = exc
    else:
        raise last_exc
    return np.concatenate(
        [res.results[c]["out"] for c in range(NCORES)], axis=1)
